# revision 14
# baseline (speedup 1.0000x reference)
"""CustomPoseLoss Trainium2 kernel.

loss = mean((pred-target)^2) + 0.5 * mean((R(pred)-R(target))^2)
where R(M) = sign(det M) * polar(M) for each 3x3 matrix (row of 9).

Implementation: closed-form polar decomposition per row, fully vectorized as
channel-plane arithmetic on the Vector/Scalar engines:
  S = M^T M, normalized by tr(S)/3; eigenvalues of S via Cardano
  (acos/cos evaluated as polynomials so only the sqrt LUT set is needed);
  W^-1 = (S + s2 I) adj(N) / det(N) with N = ssig*S + Pg*I  (Cayley-Hamilton
  inverse-sqrt);  R = sign(det) * M W^-1 / sqrt(m).
det(N) is formed from the eigenvalue product (positive, cancellation-free)
and clamped, so near-singular rows stay bounded.

Sharding: pure data parallel over 8 cores; each core reduces its shard to
[128, 2, NCHUNK] partial sums (mse, rot), host combines in float64.
"""

import numpy as np

B = 1048576
N_CORES = 8
ROWS_PER_CORE = B // N_CORES          # 131072
P = 128
ROWS_PER_PART = ROWS_PER_CORE // P    # 1024
T = 256                               # rows per partition per chunk
NCHUNK = ROWS_PER_PART // T           # 4
EPS_D = 1e-5

ACOS_A = (1.5707288, -0.2121144, 0.0742610, -0.0187293)   # A&S 4.4.45
HALF_SQRT3 = 0.8660254037844386


class Regs:
    """[128, 2, T] fp32 plane slots with explicit reuse (SBUF is capped)."""

    def __init__(self, pool, dtype):
        self.pool = pool
        self.dtype = dtype
        self.free_tags = []
        self.n = 0
        self.tag_of = {}

    def alloc(self):
        if self.free_tags:
            tag = self.free_tags.pop()
        else:
            self.n += 1
            tag = f"v{self.n}"
        tl = self.pool.tile([P, 2, T], self.dtype, tag=tag)
        self.tag_of[id(tl)] = tag
        return tl

    def free(self, *tiles):
        for tl in tiles:
            self.free_tags.append(self.tag_of.pop(id(tl)))


def _build_chunk(nc, regs, raw, praw, traw, acc_mse_col, acc_rot_col, d, dR, bias0, mybir):
    Alu = mybir.AluOpType
    Act = mybir.ActivationFunctionType

    def mul(o, a, b):
        nc.vector.tensor_tensor(out=o, in0=a, in1=b, op=Alu.mult)

    def add(o, a, b):
        nc.vector.tensor_tensor(out=o, in0=a, in1=b, op=Alu.add)

    def sub(o, a, b):
        nc.vector.tensor_tensor(out=o, in0=a, in1=b, op=Alu.subtract)

    def vs(o, a, s1, op0, s2=None, op1=None):
        if s2 is None:
            nc.vector.tensor_scalar(out=o, in0=a, scalar1=float(s1),
                                    scalar2=None, op0=getattr(Alu, op0))
        else:
            nc.vector.tensor_scalar(out=o, in0=a, scalar1=float(s1),
                                    scalar2=float(s2), op0=getattr(Alu, op0),
                                    op1=getattr(Alu, op1))

    def stt(o, a, s, b, op0, op1):
        nc.vector.scalar_tensor_tensor(out=o, in0=a, scalar=float(s), in1=b,
                                       op0=getattr(Alu, op0),
                                       op1=getattr(Alu, op1))

    def act(o, a, func, scale=1.0, accum_out=None):
        nc.scalar.activation(out=o, in_=a, func=getattr(Act, func),
                             bias=bias0[:, 0:1], scale=float(scale),
                             accum_out=accum_out)

    A = regs.alloc

    def flat(ap):
        return ap.rearrange("p z n -> p (z n)")

    def recip(o, a):
        nc.vector.reciprocal(out=o, in_=a)

    # ---- mse: d = pred - target; accumulate sum(d^2) ----
    sub(d, praw, traw)
    act(d, d, "Square", accum_out=acc_mse_col)

    # Stage DMA tiles into the combined tile on DVE: the Activation engine can
    # encode only ONE sync wait, so it must never read DMA-produced tiles
    # (two HW DMA queue semaphores) directly.
    nc.vector.tensor_copy(out=raw[:, 0, :], in_=praw)
    nc.vector.tensor_copy(out=raw[:, 1, :], in_=traw)

    rv = raw.rearrange("p z (n c) -> p z n c", c=9)
    x = [rv[:, :, :, c] for c in range(9)]      # [128, 2, T] strided views

    # ---- S = M^T M ----
    sdiag = []
    for i in range(3):
        xs0, xs1, xs2 = A(), A(), A()
        act(xs0, x[i], "Square")
        act(xs1, x[i + 3], "Square")
        act(xs2, x[i + 6], "Square")
        sd = A()
        add(sd, xs0, xs1); add(sd, sd, xs2)
        regs.free(xs0, xs1, xs2)
        sdiag.append(sd)
    s00, s11, s22 = sdiag

    def dot3(ia, ib):
        o, tmp = A(), A()
        mul(o, x[ia[0]], x[ib[0]])
        mul(tmp, x[ia[1]], x[ib[1]]); add(o, o, tmp)
        mul(tmp, x[ia[2]], x[ib[2]]); add(o, o, tmp)
        regs.free(tmp)
        return o
    s01 = dot3((0, 3, 6), (1, 4, 7))
    s02 = dot3((0, 3, 6), (2, 5, 8))
    s12 = dot3((1, 4, 7), (2, 5, 8))

    # ---- det(M) ----
    cA, cB, det, tmp = A(), A(), A(), A()
    mul(cA, x[4], x[8]); mul(cB, x[5], x[7]); sub(cA, cA, cB)
    mul(det, x[0], cA)
    mul(cA, x[5], x[6]); mul(cB, x[3], x[8]); sub(cA, cA, cB)
    mul(tmp, x[1], cA); add(det, det, tmp)
    mul(cA, x[3], x[7]); mul(cB, x[4], x[6]); sub(cA, cA, cB)
    mul(tmp, x[2], cA); add(det, det, tmp)
    regs.free(cA, cB, tmp)

    # ---- normalize S by tr/3 ----
    tr, q = A(), A()
    add(tr, s00, s11); add(tr, tr, s22)
    vs(tr, tr, 1e-20, "max")
    recip(q, tr)
    vs(q, q, 3.0, "mult")
    for s_ in (s00, s01, s02, s11, s12, s22):
        mul(s_, s_, q)
    dm2, q2, dets, Pg, sqm = A(), A(), A(), A(), A()
    act(dm2, det, "Square")
    act(q2, q, "Square")
    mul(q2, q2, q)                 # q^3
    mul(dets, dm2, q2)             # det(S-hat)
    act(Pg, dets, "Sqrt")
    act(sqm, tr, "Sqrt", scale=1.0/3.0)
    regs.free(dm2, q2, tr, q)

    # ---- tr(S^2), p ----
    u1, u2, qs = A(), A(), A()
    act(qs, s00, "Square")
    act(u1, s11, "Square"); add(u1, u1, qs)
    act(qs, s22, "Square"); add(u1, u1, qs)
    act(u2, s01, "Square")
    act(qs, s02, "Square"); add(u2, u2, qs)
    act(qs, s12, "Square"); add(u2, u2, qs)
    trS2 = A()
    stt(trS2, u2, 2.0, u1, "mult", "add")
    regs.free(u1, u2, qs)
    p, p3, u = A(), A(), A()
    vs(p3, trS2, -3.0, "add", 0.0, "max")        # trK2
    act(p, p3, "Sqrt", scale=1.0/6.0)
    act(p3, p, "Square")
    mul(p3, p3, p)
    vs(p3, p3, 1e-30, "max")
    recip(u, p3)
    regs.free(p3)

    # ---- arg = clamp(detK/(2 p^3), -1, 1) ----
    detK, arg = A(), A()
    stt(detK, trS2, 0.5, dets, "mult", "add")
    vs(detK, detK, -2.5, "add")
    mul(arg, detK, u)
    vs(arg, arg, 0.5, "mult", 1.0, "min")
    vs(arg, arg, -1.0, "max")
    regs.free(detK, u, trS2, dets)

    # ---- th3 = acos(arg): A&S 4.4.45 polynomial ----
    y, om, h, sg_a, th3 = A(), A(), A(), A(), A()
    act(y, arg, "Abs")
    vs(om, y, -1.0, "mult", 1.0, "add")
    act(om, om, "Sqrt")                          # sqrt(1-|arg|)
    vs(h, y, ACOS_A[3], "mult", ACOS_A[2], "add")
    mul(h, h, y); vs(h, h, ACOS_A[1], "add")
    mul(h, h, y); vs(h, h, ACOS_A[0], "add")
    mul(h, h, om)                                # acos(|arg|)
    act(sg_a, arg, "Sign")
    vs(om, sg_a, -np.pi/2, "mult", np.pi/2, "add")
    mul(th3, sg_a, h); add(th3, th3, om)
    regs.free(y, om, h, sg_a, arg)

    # ---- cos((th3 + 2 pi k)/3) via polynomials ----
    z, c0p = A(), A()
    act(z, th3, "Square", scale=1.0/3.0)
    vs(c0p, z, 1.0/40320.0, "mult", -1.0/720.0, "add")
    mul(c0p, c0p, z); vs(c0p, c0p, 1.0/24.0, "add")
    mul(c0p, c0p, z); vs(c0p, c0p, -0.5, "add")
    mul(c0p, c0p, z); vs(c0p, c0p, 1.0, "add")
    regs.free(z, th3)
    s0, uc1, c1p, c2p = A(), A(), A(), A()
    act(s0, c0p, "Square")
    vs(s0, s0, -1.0, "mult", 1.0, "add")
    vs(s0, s0, 0.0, "max")
    act(s0, s0, "Sqrt")                          # sin(th3/3)
    vs(uc1, c0p, -0.5, "mult")
    vs(s0, s0, HALF_SQRT3, "mult")
    sub(c1p, uc1, s0)
    add(c2p, uc1, s0)
    regs.free(s0, uc1)

    # ---- eigenvalues and their square roots ----
    tp = A()
    vs(tp, p, 2.0, "mult")
    regs.free(p)
    lam, g = [], []
    for ck in (c0p, c1p, c2p):
        lk, gk = A(), A()
        mul(lk, tp, ck)
        vs(lk, lk, 1.0, "add", 0.0, "max")
        act(gk, lk, "Sqrt")
        lam.append(lk); g.append(gk)
    regs.free(tp, c0p, c1p, c2p)

    g01, ssig, s2i, tmp2 = A(), A(), A(), A()
    add(g01, g[0], g[1])
    add(ssig, g01, g[2])
    mul(s2i, g[0], g[1]); mul(tmp2, g[2], g01); add(s2i, s2i, tmp2)
    regs.free(g01, tmp2, *g)

    # ---- detN = prod(ssig*lam_k + Pg); w = sign(det)/(sqrt(m)*detN) ----
    detN, nuk, w = A(), A(), A()
    mul(detN, ssig, lam[0]); add(detN, detN, Pg)
    mul(nuk, ssig, lam[1]); add(nuk, nuk, Pg)
    mul(detN, detN, nuk)
    mul(nuk, ssig, lam[2]); add(nuk, nuk, Pg)
    mul(detN, detN, nuk)
    vs(detN, detN, EPS_D, "max")
    mul(detN, detN, sqm)
    recip(w, detN)
    act(nuk, det, "Sign")
    mul(w, w, nuk)
    regs.free(detN, nuk, det, sqm, *lam)

    # ---- N = ssig*S + Pg*I and adj(N) ----
    n00, n01, n02, n11, n12, n22 = (A() for _ in range(6))
    mul(n00, ssig, s00); add(n00, n00, Pg)
    mul(n11, ssig, s11); add(n11, n11, Pg)
    mul(n22, ssig, s22); add(n22, n22, Pg)
    mul(n01, ssig, s01)
    mul(n02, ssig, s02)
    mul(n12, ssig, s12)
    regs.free(ssig, Pg)

    tmp3 = A()

    def cof(a, b, c, dd):
        o = A()
        mul(o, a, b); mul(tmp3, c, dd); sub(o, o, tmp3)
        return o
    a00 = cof(n11, n22, n12, n12)
    a01 = cof(n02, n12, n01, n22)
    a02 = cof(n01, n12, n02, n11)
    a11 = cof(n00, n22, n02, n02)
    a12 = cof(n01, n02, n00, n12)
    a22 = cof(n00, n11, n01, n01)
    regs.free(tmp3, n00, n01, n02, n11, n12, n22)

    # ---- A' diag; T1 = A' adjN; T2 = w*T1 ----
    b00, b11, b22 = A(), A(), A()
    add(b00, s00, s2i)
    add(b11, s11, s2i)
    add(b22, s22, s2i)
    regs.free(s2i, s00, s11, s22)

    tmp4 = A()

    def mm3(r0, r1, r2, k0, k1, k2, o=None):
        o = o or A()
        mul(o, r0, k0)
        mul(tmp4, r1, k1); add(o, o, tmp4)
        mul(tmp4, r2, k2); add(o, o, tmp4)
        return o
    t00 = mm3(b00, s01, s02, a00, a01, a02)
    t01 = mm3(b00, s01, s02, a01, a11, a12)
    t02 = mm3(b00, s01, s02, a02, a12, a22)
    t11 = mm3(s01, b11, s12, a01, a11, a12)
    t12 = mm3(s01, b11, s12, a02, a12, a22)
    t22 = mm3(s02, s12, b22, a02, a12, a22)
    regs.free(b00, b11, b22, s01, s02, s12, a00, a01, a02, a11, a12, a22)
    for t_ in (t00, t01, t02, t11, t12, t22):
        mul(t_, t_, w)
    regs.free(w)
    T2 = [[t00, t01, t02], [t01, t11, t12], [t02, t12, t22]]

    # ---- R = M*T2; dR = R_pred - R_target; accumulate sum(dR^2) ----
    Rc = A()
    for i in range(3):
        for j in range(3):
            mm3(x[3*i], x[3*i+1], x[3*i+2],
                T2[0][j], T2[1][j], T2[2][j], o=Rc)
            sub(dR[:, 3*i+j, :], Rc[:, 0, :], Rc[:, 1, :])
    regs.free(Rc, tmp4, t00, t01, t02, t11, t12, t22)
    dRf = dR.rearrange("p c n -> p (c n)")
    act(dRf, dRf, "Square", accum_out=acc_rot_col)


def _build_nc():
    import concourse.bass as bass
    import concourse.tile as tile
    from concourse import mybir

    f32 = mybir.dt.float32
    nc = bass.Bass()
    pred = nc.dram_tensor("pred", [ROWS_PER_CORE, 9], f32, kind="ExternalInput")
    targ = nc.dram_tensor("target", [ROWS_PER_CORE, 9], f32, kind="ExternalInput")
    out = nc.dram_tensor("partials", [P, 2 * NCHUNK], f32, kind="ExternalOutput")

    predv = pred.rearrange("(p n) c -> p n c", p=P)    # [128, 1024, 9]
    targv = targ.rearrange("(p n) c -> p n c", p=P)

    with tile.TileContext(nc) as tc:
        with (
            tc.tile_pool(name="raw", bufs=1) as rawp,
            tc.tile_pool(name="pl", bufs=1) as pl,
            tc.tile_pool(name="acc", bufs=1) as accp,
        ):
            acc = accp.tile([P, 2 * NCHUNK], f32, tag="acc")
            bias0 = accp.tile([P, 1], f32, tag="bias0")
            nc.vector.memset(bias0, 0.0)
            regs = Regs(pl, f32)
            # Preload the whole per-core shard once: avoids recycled DMA
            # buffers (the DMA pseudo-instruction can encode only one wait).
            praw_all = rawp.tile([P, ROWS_PER_PART * 9], f32, tag="praw")
            traw_all = rawp.tile([P, ROWS_PER_PART * 9], f32, tag="traw")
            nc.sync.dma_start(out=praw_all, in_=predv)
            nc.sync.dma_start(out=traw_all, in_=targv)
            for k in range(NCHUNK):
                praw = praw_all[:, k*T*9:(k+1)*T*9]
                traw = traw_all[:, k*T*9:(k+1)*T*9]
                raw = pl.tile([P, 2, T * 9], f32, tag="raw")
                d = pl.tile([P, 9 * T], f32, tag="d")
                dR = pl.tile([P, 9, T], f32, tag="dR")
                _build_chunk(nc, regs, raw, praw, traw,
                             acc[:, k:k+1], acc[:, NCHUNK+k:NCHUNK+k+1],
                             d, dR, bias0, mybir)
            nc.sync.dma_start(out=out[:, :], in_=acc)
    return nc


def _elide_implied_waits(nc):
    """Drop semaphore waits already implied by program order or transitively
    by earlier waits (vector-clock propagation).  Tile's per-instruction wait
    emission is not transitively minimal, and walrus can encode only one sync
    wait on Activation/DMA instructions (and ~4 on control instructions), so
    the redundant waits both break codegen and waste sequencer time.

    Model: each semaphore s carries a snapshot VC at every increment value;
    an engine's observed VC advances via its own instruction stream and via
    the snapshots of the waits it executes.  A wait (s >= v) is dropped iff
    the engine's observed VC already dominates it.  Unknown update modes
    disable elision for that semaphore (conservative).
    """
    join = lambda a, b: {k: max(a.get(k, 0), b.get(k, 0)) for k in set(a) | set(b)}
    sem_val = {}        # sem name -> current value
    sem_snap = {}       # sem name -> list of (value, VC) snapshots
    eng_vc = {}         # engine name -> observed VC
    unsafe = set()      # sems with non-increment updates
    n_drop = 0
    for f in nc.m.functions:
        for bb in f.blocks:
            for ins in bb.instructions:
                eng = str(ins.engine)
                vc = dict(eng_vc.get(eng, {}))
                si = ins.sync_info
                waits = list(si.on_wait) if si is not None and si.on_wait else []
                kept = []
                for w in waits:
                    s, v = w.ant_name, w.wait_value
                    if w.wait_mode != "sem-ge-imm" or s in unsafe:
                        kept.append(w)
                        continue
                    if vc.get(s, 0) >= v:
                        n_drop += 1
                        continue
                    if sem_val.get(s, 0) < v:
                        # increment not yet seen in emission order; keep and
                        # learn nothing (conservative)
                        kept.append(w)
                        continue
                    kept.append(w)
                    snap = {}
                    for sv, svc in sem_snap.get(s, ()):
                        if sv <= v:
                            snap = svc
                        else:
                            break
                    vc = join(vc, snap)
                    vc[s] = max(vc.get(s, 0), v)
                if si is not None and len(kept) != len(waits):
                    si.on_wait = kept
                # apply this instruction's increments
                ups = si.on_update if si is not None and si.on_update else []
                for u in ups:
                    s = u.ant_name
                    if u.update_mode not in ("sem-inc", "sem-add-imm"):
                        unsafe.add(s)
                        continue
                    nv = sem_val.get(s, 0) + (u.update_value or 1)
                    sem_val[s] = nv
                    lst = sem_snap.setdefault(s, [])
                    prev = lst[-1][1] if lst else {}
                    lst.append((nv, join(prev, vc)))
                    # Engine-sem increments fire when the instruction
                    # completes, and the engine is sequential, so later
                    # instructions on this engine observe them.  DMA-queue
                    # increments fire asynchronously at transfer completion:
                    # the issuing engine must NOT absorb those.
                    if "DMA" not in s:
                        vc[s] = max(vc.get(s, 0), nv)
                eng_vc[eng] = vc
    return n_drop


_NC_CACHE = None


def kernel(pred: np.ndarray, target: np.ndarray) -> np.ndarray:
    global _NC_CACHE
    from concourse.bass_utils import run_bass_kernel_spmd

    pred = np.ascontiguousarray(np.asarray(pred, dtype=np.float32))
    target = np.ascontiguousarray(np.asarray(target, dtype=np.float32))
    assert pred.shape == (B, 9) and target.shape == (B, 9)

    if _NC_CACHE is None:
        _NC_CACHE = _build_nc()
        _elide_implied_waits(_NC_CACHE)
    nc = _NC_CACHE

    ps = pred.reshape(N_CORES, ROWS_PER_CORE, 9)
    ts = target.reshape(N_CORES, ROWS_PER_CORE, 9)
    in_maps = [{"pred": ps[i], "target": ts[i]} for i in range(N_CORES)]
    res = run_bass_kernel_spmd(nc, in_maps, core_ids=list(range(N_CORES)))
    globals()["_LAST_RESULT"] = res

    mse_sum = 0.0
    rot_sum = 0.0
    for r in res.results:
        part = np.asarray(r["partials"], dtype=np.float64)
        mse_sum += part[:, :NCHUNK].sum()
        rot_sum += part[:, NCHUNK:].sum()
    n = float(B * 9)
    return np.float32(mse_sum / n + 0.5 * (rot_sum / n))


# revision 19
# speedup vs baseline: 1.4844x; 1.4844x over previous
"""CustomPoseLoss Trainium2 kernel.

loss = mean((pred-target)^2) + 0.5 * mean((R(pred)-R(target))^2)
where R(M) = sign(det M) * polar(M) for each 3x3 matrix (row of 9).

Implementation: closed-form polar decomposition per row, fully vectorized as
channel-plane arithmetic on the Vector/Scalar engines:
  S = M^T M, normalized by tr(S)/3; eigenvalues of S via Cardano
  (acos/cos evaluated as polynomials so only the sqrt LUT set is needed);
  W^-1 = (S + s2 I) adj(N) / det(N) with N = ssig*S + Pg*I  (Cayley-Hamilton
  inverse-sqrt);  R = sign(det) * M W^-1 / sqrt(m).
det(N) is formed from the eigenvalue product (positive, cancellation-free)
and clamped, so near-singular rows stay bounded.

Sharding: pure data parallel over 8 cores; each core reduces its shard to
[128, 2, NCHUNK] partial sums (mse, rot), host combines in float64.
"""

import numpy as np

B = 1048576
N_CORES = 8
ROWS_PER_CORE = B // N_CORES          # 131072
P = 128
ROWS_PER_PART = ROWS_PER_CORE // P    # 1024
T = 256                               # rows per partition per chunk
NCHUNK = ROWS_PER_PART // T           # 4
EPS_D = 1e-5

ACOS_A = (1.5707288, -0.2121144, 0.0742610, -0.0187293)   # A&S 4.4.45
HALF_SQRT3 = 0.8660254037844386


class Regs:
    """[128, 2, T] fp32 plane slots with explicit reuse (SBUF is capped)."""

    def __init__(self, pool, dtype, prefix="v", shape=None):
        self.pool = pool
        self.dtype = dtype
        self.prefix = prefix
        self.shape = shape or [P, 2 * T]
        self.free_tags = []
        self.n = 0
        self.tag_of = {}

    def alloc(self):
        if self.free_tags:
            tag = self.free_tags.pop()
        else:
            self.n += 1
            tag = f"{self.prefix}{self.n}"
        tl = self.pool.tile(self.shape, self.dtype, tag=tag)
        self.tag_of[id(tl)] = tag
        return tl

    def free(self, *tiles):
        for tl in tiles:
            self.free_tags.append(self.tag_of.pop(id(tl)))


LN3 = float(np.log(3.0))
LN6 = float(np.log(6.0))
LN2 = float(np.log(2.0))
EPS_W = 2e-3


def _build_chunk(nc, regs, regs16, praw, traw, acc_mse_col, acc_rot_col,
                 X, D, Sm, QS, Nm, Am, T1m, RT, dR, bias0, mybir):
    Alu = mybir.AluOpType
    Act = mybir.ActivationFunctionType
    L = 2 * T

    def mul(o, a, b):
        nc.vector.tensor_tensor(out=o, in0=a, in1=b, op=Alu.mult)

    def add(o, a, b):
        nc.vector.tensor_tensor(out=o, in0=a, in1=b, op=Alu.add)

    def sub(o, a, b):
        nc.vector.tensor_tensor(out=o, in0=a, in1=b, op=Alu.subtract)

    def vs(o, a, s1, op0, s2=None, op1=None):
        if s2 is None:
            nc.vector.tensor_scalar(out=o, in0=a, scalar1=float(s1),
                                    scalar2=None, op0=getattr(Alu, op0))
        else:
            nc.vector.tensor_scalar(out=o, in0=a, scalar1=float(s1),
                                    scalar2=float(s2), op0=getattr(Alu, op0),
                                    op1=getattr(Alu, op1))

    def stt(o, a, s, b, op0, op1):
        nc.vector.scalar_tensor_tensor(out=o, in0=a, scalar=float(s), in1=b,
                                       op0=getattr(Alu, op0),
                                       op1=getattr(Alu, op1))

    def act(o, a, func, scale=1.0, bias=None, accum_out=None):
        if func == "Copy":
            nc.scalar.activation(out=o, in_=a, func=Act.Copy, bias=0.0,
                                 scale=float(scale), accum_out=accum_out)
        else:
            nc.scalar.activation(out=o, in_=a, func=getattr(Act, func),
                                 bias=bias0[:, 0:1] if bias is None else bias,
                                 scale=float(scale), accum_out=accum_out)

    def bc(plane, k):
        # broadcast [P, L] plane across k sub-planes -> [P, k, L]
        return bass_mod.AP(tensor=plane.tensor, offset=plane.offset,
                           ap=[plane.ap[0], [0, k], plane.ap[1]])

    A = regs.alloc          # fp32 [P, L] planes
    H = regs16.alloc        # fp16 [P, L] planes

    # ---- cast+deinterleave both inputs into X[P, 9, 2T] (f16) ----
    rvp = praw.rearrange("p (n c) -> p n c", c=9)
    rvt = traw.rearrange("p (n c) -> p n c", c=9)
    xin_p = bass_mod.AP(tensor=rvp.tensor, offset=rvp.offset,
                        ap=[rvp.ap[0], rvp.ap[2], rvp.ap[1]])
    xin_t = bass_mod.AP(tensor=rvt.tensor, offset=rvt.offset,
                        ap=[rvt.ap[0], rvt.ap[2], rvt.ap[1]])
    act(X[:, :, 0:T], xin_p, "Copy")
    act(X[:, :, T:L], xin_t, "Copy")
    x = [X[:, c, :] for c in range(9)]          # [P, L] f16 unit-stride

    # ---- mse: D = pred - target (f16), accum sum(D^2) on ACT ----
    sub(D, X[:, :, 0:T], X[:, :, T:L])
    Df = D.rearrange("p c n -> p (c n)")
    act(Df, Df, "Square", accum_out=acc_mse_col)

    # ---- S = M^T M (f16): order [s00,s11,s22,s01,s02,s12] ----
    tmp16 = H()
    for i in range(3):
        sd = Sm[:, i, :]
        mul(sd, x[i], x[i])
        mul(tmp16, x[i+3], x[i+3]); add(sd, sd, tmp16)
        mul(tmp16, x[i+6], x[i+6]); add(sd, sd, tmp16)
    for oi, (ia, ib) in enumerate((((0,3,6),(1,4,7)), ((0,3,6),(2,5,8)),
                                   ((1,4,7),(2,5,8)))):
        so = Sm[:, 3+oi, :]
        mul(so, x[ia[0]], x[ib[0]])
        mul(tmp16, x[ia[1]], x[ib[1]]); add(so, so, tmp16)
        mul(tmp16, x[ia[2]], x[ib[2]]); add(so, so, tmp16)
    # tr and normalization scale q = 3/tr (ln domain)
    tr16 = H()
    add(tr16, Sm[:, 0, :], Sm[:, 1, :]); add(tr16, tr16, Sm[:, 2, :])
    vs(tr16, tr16, 6e-5, "max")
    lnt = A(); act(lnt, tr16, "Ln")
    q16 = H(); act(q16, lnt, "Exp", scale=-1.0, bias=_c(nc, LN3))
    regs16.free(tr16)
    nc.vector.tensor_tensor(out=Sm[:, :, :], in0=Sm[:, :, :], in1=bc(q16, 6),
                            op=Alu.mult)
    regs16.free(q16)

    # ---- det(M) fp32 from raw (strided channel views) ----
    xr = [None] * 9
    for c in range(9):
        ap_p = rvp[:, :, c]
        ap_t = rvt[:, :, c]
        xr[c] = (ap_p, ap_t)
    cA, cB, det, tmpd = A(), A(), A(), A()
    def rmul(o, i, j):
        # o[:, :T] = pred_ch_i*pred_ch_j ; o[:, T:] = target halves
        mul(o[:, 0:T], xr[i][0], xr[j][0])
        mul(o[:, T:L], xr[i][1], xr[j][1])
    def rmul2(o, i, co):
        mul(o[:, 0:T], xr[i][0], co[:, 0:T])
        mul(o[:, T:L], xr[i][1], co[:, T:L])
    rmul(cA, 4, 8); rmul(cB, 5, 7); sub(cA, cA, cB)
    rmul2(det, 0, cA)
    rmul(cA, 5, 6); rmul(cB, 3, 8); sub(cA, cA, cB)
    rmul2(tmpd, 1, cA); add(det, det, tmpd)
    rmul(cA, 3, 7); rmul(cB, 4, 6); sub(cA, cA, cB)
    rmul2(tmpd, 2, cA); add(det, det, tmpd)
    regs.free(cA)
    sgd = A(); act(sgd, det, "Sign")
    lnad = A(); act(cB, det, "Abs")
    act(lnad, cB, "Ln")
    regs.free(cB, det, tmpd)
    # Pg = exp(lnad + 1.5*(ln3 - lnt));  dets = Pg^2
    lnPg = A()
    stt(lnPg, lnt, -1.5, lnad, "mult", "add")
    regs.free(lnad)
    Pg32 = A(); act(Pg32, lnPg, "Exp", scale=1.0, bias=_c(nc, 1.5 * LN3))
    Pg16 = H(); act(Pg16, Pg32, "Copy")
    dets = A(); act(dets, Pg32, "Square")
    regs.free(lnPg)

    # ---- tr(S^2) fp32 from normalized f16 S ----
    act(QS, Sm, "Square")
    u1, u2 = A(), A()
    add(u1, QS[:, 0, :], QS[:, 1, :]); add(u1, u1, QS[:, 2, :])
    add(u2, QS[:, 3, :], QS[:, 4, :]); add(u2, u2, QS[:, 5, :])
    trS2 = A()
    stt(trS2, u2, 2.0, u1, "mult", "add")
    # p and 1/(2 p^3) via ln/exp
    trK2, p, ip3h = u1, A(), u2            # reuse u1/u2 slots
    vs(trK2, trS2, -3.0, "add", 1e-30, "max")
    lnk = A(); act(lnk, trK2, "Ln")
    act(p, lnk, "Exp", scale=0.5, bias=_c(nc, -0.5 * LN6))
    act(ip3h, lnk, "Exp", scale=-1.5, bias=_c(nc, 1.5 * LN6 - LN2))
    vs(ip3h, ip3h, 1e30, "min")
    regs.free(lnk)
    # arg
    detK, arg = A(), A()
    stt(detK, trS2, 0.5, dets, "mult", "add")
    vs(detK, detK, -2.5, "add")
    mul(arg, detK, ip3h)
    vs(arg, arg, 1.0, "min", -1.0, "max")
    regs.free(detK, trS2, dets, u2)   # u2 == ip3h
    # ---- th3 = acos(arg) ----
    y, om, h = A(), A(), A()
    act(y, arg, "Abs")
    vs(om, y, -1.0, "mult", 1.0, "add")
    lnom = A(); act(lnom, om, "Ln")
    act(om, lnom, "Exp", scale=0.5)              # sqrt(1-y)
    regs.free(lnom)
    vs(h, y, ACOS_A[3], "mult", ACOS_A[2], "add")
    mul(h, h, y); vs(h, h, ACOS_A[1], "add")
    mul(h, h, y); vs(h, h, ACOS_A[0], "add")
    mul(h, h, om)
    sg_a, th3 = y, om                            # reuse slots
    act(sg_a, arg, "Sign")
    vs(arg, sg_a, -np.pi/2, "mult", np.pi/2, "add")
    mul(th3, sg_a, h); add(th3, th3, arg)
    regs.free(h, arg, y)    # th3 == om stays
    # ---- cos((th3+2pik)/3) ----
    z, c0p = A(), A()
    act(z, th3, "Square", scale=1.0/3.0)
    vs(c0p, z, 1.0/40320.0, "mult", -1.0/720.0, "add")
    mul(c0p, c0p, z); vs(c0p, c0p, 1.0/24.0, "add")
    mul(c0p, c0p, z); vs(c0p, c0p, -0.5, "add")
    mul(c0p, c0p, z); vs(c0p, c0p, 1.0, "add")
    regs.free(z, om)   # om == th3
    s0, uc1, c1p, c2p = A(), A(), A(), A()
    act(s0, c0p, "Square")
    vs(s0, s0, -1.0, "mult", 1.0, "add")
    vs(s0, s0, 0.0, "max")
    lns = A(); act(lns, s0, "Ln")
    act(s0, lns, "Exp", scale=0.5)
    regs.free(lns)
    vs(uc1, c0p, -0.5, "mult")
    vs(s0, s0, HALF_SQRT3, "mult")
    sub(c1p, uc1, s0)
    add(c2p, uc1, s0)
    regs.free(s0, uc1)
    # ---- lambda_k, g_k = sqrt(lambda_k) ----
    tp = A()
    vs(tp, p, 2.0, "mult")
    regs.free(p)
    lam, g = [], []
    for ck in (c0p, c1p, c2p):
        lk, gk, lnl = A(), A(), A()
        mul(lk, tp, ck)
        vs(lk, lk, 1.0, "add", 1e-35, "max")
        act(lnl, lk, "Ln")
        act(gk, lnl, "Exp", scale=0.5)
        regs.free(lnl)
        lam.append(lk); g.append(gk)
    regs.free(tp, c0p, c1p, c2p)
    g01, ssig, s2i, tmp2 = A(), A(), A(), A()
    add(g01, g[0], g[1])
    add(ssig, g01, g[2])
    mul(s2i, g[0], g[1]); mul(tmp2, g[2], g01); add(s2i, s2i, tmp2)
    regs.free(g01, tmp2, *g)
    # ---- w = sign/(sqrt(m)*detN) via ln domain, clamped ----
    t_, nuk, lnn = A(), A(), A()
    mul(nuk, ssig, lam[0]); add(nuk, nuk, Pg32)
    act(t_, nuk, "Ln")
    mul(nuk, ssig, lam[1]); add(nuk, nuk, Pg32)
    act(lnn, nuk, "Ln"); add(t_, t_, lnn)
    mul(nuk, ssig, lam[2]); add(nuk, nuk, Pg32)
    act(lnn, nuk, "Ln"); add(t_, t_, lnn)
    stt(t_, lnt, 0.5, t_, "mult", "add")
    vs(t_, t_, float(np.log(EPS_W) + 0.5*LN3), "max")
    wmag = lnn                                  # reuse
    act(wmag, t_, "Exp", scale=-1.0, bias=_c(nc, 0.5 * LN3))
    w16 = H()
    mul(w16, wmag, sgd)
    regs.free(t_, nuk, lnn, sgd, lnt, *lam)
    ssig16, s2i16 = H(), H()
    act(ssig16, ssig, "Copy")
    act(s2i16, s2i, "Copy")
    regs.free(ssig, s2i, Pg32)

    # ---- N = ssig*S + Pg*I (f16, batched) ----
    nc.vector.tensor_tensor(out=Nm[:, :, :], in0=Sm[:, :, :],
                            in1=bc(ssig16, 6), op=Alu.mult)
    nc.vector.tensor_tensor(out=Nm[:, 0:3, :], in0=Nm[:, 0:3, :],
                            in1=bc(Pg16, 3), op=Alu.add)
    regs16.free(Pg16, ssig16)
    # A' diagonal (Am = S_diag + s2)
    nc.vector.tensor_tensor(out=Am[:, :, :], in0=Sm[:, 0:3, :],
                            in1=bc(s2i16, 3), op=Alu.add)
    regs16.free(s2i16)
    # ---- adj(N) (f16) -> stored into QS? no: reuse Nm? need both. use T1m? no.
    n00, n11, n22 = (Nm[:, i, :] for i in range(3))
    n01, n02, n12 = (Nm[:, i, :] for i in range(3, 6))
    aj = [H() for _ in range(6)]
    a00, a01, a02, a11, a12, a22 = aj
    def cof(o, a, b, c, dd):
        mul(o, a, b); mul(tmp16, c, dd); sub(o, o, tmp16)
    cof(a00, n11, n22, n12, n12)
    cof(a01, n02, n12, n01, n22)
    cof(a02, n01, n12, n02, n11)
    cof(a11, n00, n22, n02, n02)
    cof(a12, n01, n02, n00, n12)
    cof(a22, n00, n11, n01, n01)
    # ---- T1 = A' adjN (f16); rows of A': (b0,s01,s02),(s01,b1,s12),(s02,s12,b2)
    b0, b1, b2 = (Am[:, i, :] for i in range(3))
    s01p, s02p, s12p = Sm[:, 3, :], Sm[:, 4, :], Sm[:, 5, :]
    def mm3(o, r0, r1, r2, k0, k1, k2):
        mul(o, r0, k0)
        mul(tmp16, r1, k1); add(o, o, tmp16)
        mul(tmp16, r2, k2); add(o, o, tmp16)
    mm3(T1m[:, 0, :], b0, s01p, s02p, a00, a01, a02)
    mm3(T1m[:, 1, :], b0, s01p, s02p, a01, a11, a12)
    mm3(T1m[:, 2, :], b0, s01p, s02p, a02, a12, a22)
    mm3(T1m[:, 3, :], s01p, b1, s12p, a01, a11, a12)
    mm3(T1m[:, 4, :], s01p, b1, s12p, a02, a12, a22)
    mm3(T1m[:, 5, :], s02p, s12p, b2, a02, a12, a22)
    regs16.free(*aj)
    # ---- T2 = clamp(w*T1, +-3e3) ----
    nc.vector.tensor_tensor(out=T1m[:, :, :], in0=T1m[:, :, :],
                            in1=bc(w16, 6), op=Alu.mult)
    T1f = T1m.rearrange("p c n -> p (c n)")
    nc.vector.tensor_scalar(out=T1f, in0=T1f, scalar1=3000.0, scalar2=-3000.0,
                            op0=Alu.min, op1=Alu.max)
    regs16.free(w16)
    t00, t01, t02 = T1m[:, 0, :], T1m[:, 1, :], T1m[:, 2, :]
    t11, t12, t22 = T1m[:, 3, :], T1m[:, 4, :], T1m[:, 5, :]
    T2 = [[t00, t01, t02], [t01, t11, t12], [t02, t12, t22]]
    # ---- R = M*T2 (f16), clamp, dR, accumulate ----
    for i in range(3):
        for j in range(3):
            mm3(RT[:, 3*i+j, :], x[3*i], x[3*i+1], x[3*i+2],
                T2[0][j], T2[1][j], T2[2][j])
    RTf = RT.rearrange("p c n -> p (c n)")
    nc.vector.tensor_scalar(out=RTf, in0=RTf, scalar1=8.0, scalar2=-8.0,
                            op0=Alu.min, op1=Alu.max)
    sub(dR, RT[:, :, 0:T], RT[:, :, T:L])
    dRf = dR.rearrange("p c n -> p (c n)")
    act(dRf, dRf, "Square", accum_out=acc_rot_col)
    regs16.free(tmp16)



_CONST_STATE = {}
bass_mod = None


def _c(nc, v):
    """[P,1] fp32 constant AP, DVE-memset once (keeps ACT single-wait)."""
    key = float(np.float32(v))
    consts = _CONST_STATE.setdefault(id(nc), {})
    if key not in consts:
        pool = _CONST_STATE[(id(nc), "pool")]
        from concourse import mybir
        t = pool.tile([P, 1], mybir.dt.float32, tag=f"c{len(consts)}")
        nc.vector.memset(t, key)
        consts[key] = t
    return consts[key][:, 0:1]


def _build_nc():
    global bass_mod
    import concourse.bass as bass
    import concourse.tile as tile
    from concourse import mybir
    bass_mod = bass

    f32 = mybir.dt.float32
    f16 = mybir.dt.float16
    nc = bass.Bass()
    pred = nc.dram_tensor("pred", [ROWS_PER_CORE, 9], f32, kind="ExternalInput")
    targ = nc.dram_tensor("target", [ROWS_PER_CORE, 9], f32, kind="ExternalInput")
    out = nc.dram_tensor("partials", [P, 2 * NCHUNK], f32, kind="ExternalOutput")

    predv = pred.rearrange("(p n) c -> p n c", p=P)    # [128, 1024, 9]
    targv = targ.rearrange("(p n) c -> p n c", p=P)

    with tile.TileContext(nc) as tc:
        with (
            tc.tile_pool(name="raw", bufs=1) as rawp,
            tc.tile_pool(name="pl", bufs=1) as pl,
            tc.tile_pool(name="acc", bufs=1) as accp,
        ):
            acc = accp.tile([P, 2 * NCHUNK], f32, tag="acc")
            bias0 = accp.tile([P, 1], f32, tag="bias0")
            nc.vector.memset(bias0, 0.0)
            _CONST_STATE[(id(nc), "pool")] = accp
            regs = Regs(pl, f32, prefix="v", shape=[P, 2 * T])
            regs16 = Regs(pl, f16, prefix="h", shape=[P, 2 * T])
            L = 2 * T
            praw_all = rawp.tile([P, ROWS_PER_PART * 9], f32, tag="praw")
            traw_all = rawp.tile([P, ROWS_PER_PART * 9], f32, tag="traw")
            nc.sync.dma_start(out=praw_all, in_=predv)
            nc.sync.dma_start(out=traw_all, in_=targv)
            for k in range(NCHUNK):
                praw = praw_all[:, k*T*9:(k+1)*T*9]
                traw = traw_all[:, k*T*9:(k+1)*T*9]
                X = pl.tile([P, 9, L], f16, tag="X")
                D = pl.tile([P, 9, T], f16, tag="D")
                Sm = pl.tile([P, 6, L], f16, tag="Sm")
                QS = pl.tile([P, 6, L], f32, tag="QS")
                Nm = pl.tile([P, 6, L], f16, tag="Nm")
                Am = pl.tile([P, 3, L], f16, tag="Am")
                T1m = pl.tile([P, 6, L], f16, tag="T1m")
                RT = pl.tile([P, 9, L], f16, tag="RT")
                dR = pl.tile([P, 9, T], f16, tag="dRt")
                _build_chunk(nc, regs, regs16, praw, traw,
                             acc[:, k:k+1], acc[:, NCHUNK+k:NCHUNK+k+1],
                             X, D, Sm, QS, Nm, Am, T1m, RT, dR, bias0, mybir)
            nc.sync.dma_start(out=out[:, :], in_=acc)
    return nc


def _elide_implied_waits(nc):
    """Drop semaphore waits already implied by program order or transitively
    by earlier waits (vector-clock propagation).  Tile's per-instruction wait
    emission is not transitively minimal, and walrus can encode only one sync
    wait on Activation/DMA instructions (and ~4 on control instructions), so
    the redundant waits both break codegen and waste sequencer time.

    Model: each semaphore s carries a snapshot VC at every increment value;
    an engine's observed VC advances via its own instruction stream and via
    the snapshots of the waits it executes.  A wait (s >= v) is dropped iff
    the engine's observed VC already dominates it.  Unknown update modes
    disable elision for that semaphore (conservative).
    """
    join = lambda a, b: {k: max(a.get(k, 0), b.get(k, 0)) for k in set(a) | set(b)}
    sem_val = {}        # sem name -> current value
    sem_snap = {}       # sem name -> list of (value, VC) snapshots
    eng_vc = {}         # engine name -> observed VC
    unsafe = set()      # sems with non-increment updates
    n_drop = 0
    for f in nc.m.functions:
        for bb in f.blocks:
            for ins in bb.instructions:
                eng = str(ins.engine)
                vc = dict(eng_vc.get(eng, {}))
                si = ins.sync_info
                waits = list(si.on_wait) if si is not None and si.on_wait else []
                kept = []
                for w in waits:
                    s, v = w.ant_name, w.wait_value
                    if w.wait_mode != "sem-ge-imm" or s in unsafe:
                        kept.append(w)
                        continue
                    if vc.get(s, 0) >= v:
                        n_drop += 1
                        continue
                    if sem_val.get(s, 0) < v:
                        # increment not yet seen in emission order; keep and
                        # learn nothing (conservative)
                        kept.append(w)
                        continue
                    kept.append(w)
                    snap = {}
                    for sv, svc in sem_snap.get(s, ()):
                        if sv <= v:
                            snap = svc
                        else:
                            break
                    vc = join(vc, snap)
                    vc[s] = max(vc.get(s, 0), v)
                if si is not None and len(kept) != len(waits):
                    si.on_wait = kept
                # apply this instruction's increments
                ups = si.on_update if si is not None and si.on_update else []
                for u in ups:
                    s = u.ant_name
                    if u.update_mode not in ("sem-inc", "sem-add-imm"):
                        unsafe.add(s)
                        continue
                    nv = sem_val.get(s, 0) + (u.update_value or 1)
                    sem_val[s] = nv
                    lst = sem_snap.setdefault(s, [])
                    prev = lst[-1][1] if lst else {}
                    lst.append((nv, join(prev, vc)))
                    # Engine-sem increments fire when the instruction
                    # completes, and the engine is sequential, so later
                    # instructions on this engine observe them.  DMA-queue
                    # increments fire asynchronously at transfer completion:
                    # the issuing engine must NOT absorb those.
                    if "DMA" not in s:
                        vc[s] = max(vc.get(s, 0), nv)
                eng_vc[eng] = vc
    return n_drop


_NC_CACHE = None


def kernel(pred: np.ndarray, target: np.ndarray) -> np.ndarray:
    global _NC_CACHE
    from concourse.bass_utils import run_bass_kernel_spmd

    pred = np.ascontiguousarray(np.asarray(pred, dtype=np.float32))
    target = np.ascontiguousarray(np.asarray(target, dtype=np.float32))
    assert pred.shape == (B, 9) and target.shape == (B, 9)

    if _NC_CACHE is None:
        _NC_CACHE = _build_nc()
        _elide_implied_waits(_NC_CACHE)
    nc = _NC_CACHE

    ps = pred.reshape(N_CORES, ROWS_PER_CORE, 9)
    ts = target.reshape(N_CORES, ROWS_PER_CORE, 9)
    in_maps = [{"pred": ps[i], "target": ts[i]} for i in range(N_CORES)]
    res = run_bass_kernel_spmd(nc, in_maps, core_ids=list(range(N_CORES)))
    globals()["_LAST_RESULT"] = res

    mse_sum = 0.0
    rot_sum = 0.0
    for r in res.results:
        part = np.asarray(r["partials"], dtype=np.float64)
        mse_sum += part[:, :NCHUNK].sum()
        rot_sum += part[:, NCHUNK:].sum()
    n = float(B * 9)
    return np.float32(mse_sum / n + 0.5 * (rot_sum / n))
